# revision 23
# baseline (speedup 1.0000x reference)
"""Trainium2 Bass kernel for the LoTD Sinkhorn OT loss (nn_LoTD_55619826483669).

v2 — algorithmic + scheduling rewrite of the 156us baseline.

Math (validated numerically to ~5e-6 vs the reference, incl. bf16 casts):

  Log-space Sinkhorn collapses to multiplicative Sinkhorn on
  K0' = N*exp(-2 dots / reg) (the exp(sq/reg) rank-1 factors fold into the
  scaling vectors; the a=b=1/N marginals fold into K0' as +ln(N) exp bias):

      q0 = exp(sq_t/reg);  p = 1/(K0' q0)          [ITERS=1 suffices: 3e-5]

  The loss decomposes as term1 + term2 + term3 where, ending on the
  u-product, term1 ~= sum(sq_s)/N = HID/N exactly (L2norm over tokens) and
  term2 = HID/N exactly, so term1+term2 = 2*HID/N = 128/576 is a HOST-SIDE
  CONSTANT.  Only term3 is computed on device:

      u = K0'^T p;  w = (K0' o dots)^T p;  term3 = -(2/N) sum_m w_m / u_m

  (q = 1/u never needs to be materialized: q_m*w_m = w_m/u_m.)

Layout: tokens viewed as i = 5p + b (p: partition, b: block), padded to 640.
Pad rows of K0'/K0'^T are zeroed via a -100 exp bias so every matvec stays
exact and finite; pad columns evaluate to finite junk that never contaminates
valid entries.

Schedule: 2 sample-pairs per core; pair-stacked [128,*] tiles let the
projection col-pack (tile_position via out base partition) and the K-gen
row-pack (64-contract halves), halving tensor time.  20 warm-up matmuls at
the head hold the PE HAM clock at 2.4GHz through the DMA-bound front.  The
serial wall is the 40 exp activations on ScalarE; everything else hides
under it.

Sharding: pure data parallel, 4 samples per core on 8 cores; host sums the
8 scalar partials and adds the 128/576 constant.
"""

import math

import numpy as np

import concourse.bass as bass
import concourse.mybir as mybir
import concourse.tile as tile
from concourse.bass_utils import run_bass_kernel_spmd
from concourse.vector_clock import ScopedClock

# -------- problem constants (hardcoded per the harness contract) --------
BS, CS, CT, H, W, HID = 32, 640, 768, 24, 24, 64
N = H * W                      # 576 tokens
NP = 640                       # padded tokens = 5 * 128
NB = 5                         # token blocks
REG = 0.1
LN_N = math.log(N)             # folds a=b=1/N into the kernel matrix
N_CORES = 8
SPC = BS // N_CORES            # samples per core
NPAIR = SPC // 2               # sample pairs per core
CSC = CS // 128
CTC = CT // 128
# first padded partition per block b: smallest p with 5p+b >= 576
PAD_P = [(N - b + NB - 1) // NB for b in range(NB)]
REG_FULL = ((0, 512), (512, NP))   # 640-wide streams (PSUM bank split)
REG_N = ((0, 512), (512, N))       # valid-token-only streams

F32 = mybir.dt.float32
BF16 = mybir.dt.bfloat16
AX = mybir.AxisListType.X
OP = mybir.AluOpType
AF = mybir.ActivationFunctionType

N_DUMMY = 12                   # HAM warm-up matmuls at the head


def _install_drain_fix():
    """This walrus build accepts only one sync-wait per instruction: split the
    TileContext tail-drain waits across single-wait NOPs, and split any
    scheduled instruction's multi-waits the same way."""
    def _patched(self, tick_clock, wait_clock):
        nc = self.nc
        carrier = nc.sync.nop()
        wait_clock.add_sem_waits(
            carrier.ins, ScopedClock({None: tick_clock.global_clock})
        )
        waits = list(carrier.ins.sync_info.on_wait)
        carrier.ins.sync_info.on_wait = waits[:1]
        for w in waits[1:]:
            n = nc.sync.nop()
            n.ins.sync_info = mybir.SyncInfo(on_wait=[w], on_update=[])
        nc.sync.drain()
        nc.all_engine_barrier()
        popped = nc._tile_sem_poison_stack.pop()
        assert popped is self._sem_poison
        nc.clear_and_free_semaphores(list(self.sems.allocated().values()))
        nc.all_engine_barrier()

    tile.TileContext._drain_and_barrier = _patched

    if not getattr(tile.TileContext, "_ant_split_waits", False):
        orig_add = tile.TileContext._add_instruction

        def _add_split(self, inst):
            si = inst.sync_info
            if si is not None and si.on_wait is not None and len(si.on_wait) > 1:
                waits = list(si.on_wait)
                for w in waits[:-1]:
                    nop = mybir.InstNoOp(
                        name=self.nc.get_next_instruction_name(), ins=[], outs=[])
                    nop.engine = inst.engine
                    nop.sync_info = mybir.SyncInfo(on_wait=[w], on_update=[])
                    orig_add(self, nop)
                inst.sync_info = mybir.SyncInfo(
                    on_wait=[waits[-1]], on_update=list(si.on_update or []))
            orig_add(self, inst)

        tile.TileContext._add_instruction = _add_split
        tile.TileContext._ant_split_waits = True


def build_program():
    _install_drain_fix()
    nc = bass.Bass("TRN2", target_bir_lowering=False, debug=False)

    fs_d = nc.dram_tensor("feat_s", [SPC, CS, N], BF16, kind="ExternalInput")
    ft_d = nc.dram_tensor("feat_t", [SPC, CT, N], BF16, kind="ExternalInput")
    wst_d = nc.dram_tensor("WsT", [CS, HID], BF16, kind="ExternalInput")
    wtt_d = nc.dram_tensor("WtT", [CT, HID], BF16, kind="ExternalInput")
    bs_d = nc.dram_tensor("bs", [HID], F32, kind="ExternalInput")
    bt_d = nc.dram_tensor("bt", [HID], F32, kind="ExternalInput")
    loss_d = nc.dram_tensor("loss", [1], F32, kind="ExternalOutput")

    def dmaq(i):
        return nc.sync if i % 2 == 0 else nc.scalar

    with tile.TileContext(nc) as tc:
        with (
            tc.tile_pool(name="singles", bufs=1) as singles,
            tc.tile_pool(name="feats", bufs=4) as feats,
            tc.tile_pool(name="pairs", bufs=2) as pairs,
            tc.tile_pool(name="ktiles", bufs=4) as ktp,
            tc.tile_pool(name="cols", bufs=4) as cols,
            tc.tile_pool(name="rows", bufs=4) as rows,
            tc.tile_pool(name="small", bufs=8) as small,
            tc.tile_pool(name="psXP", bufs=1, space="PSUM") as psXP,
            tc.tile_pool(name="psD", bufs=2, space="PSUM") as psD,
            tc.tile_pool(name="psB", bufs=1, space="PSUM") as psB,
        ):
            # ---- weights / biases / constants, then feature streams ----
            wst_sb = singles.tile([128, CSC, HID], BF16)
            nc.sync.dma_start(out=wst_sb, in_=wst_d.ap().rearrange("(c p) h -> p c h", p=128))
            wtt_sb = singles.tile([128, CTC, HID], BF16)
            nc.scalar.dma_start(out=wtt_sb, in_=wtt_d.ap().rearrange("(c p) h -> p c h", p=128))
            # pair-stacked biases: sample a at partitions [0:64), b at [64:128)
            bs2 = singles.tile([128, 1], F32)
            nc.sync.dma_start(out=bs2[0:HID, :], in_=bs_d.ap().rearrange("(p o) -> p o", o=1))
            nc.sync.dma_start(out=bs2[HID:128, :], in_=bs_d.ap().rearrange("(p o) -> p o", o=1))
            bt2 = singles.tile([128, 1], F32)
            nc.scalar.dma_start(out=bt2[0:HID, :], in_=bt_d.ap().rearrange("(p o) -> p o", o=1))
            nc.scalar.dma_start(out=bt2[HID:128, :], in_=bt_d.ap().rearrange("(p o) -> p o", o=1))
            # feature tiles: each sample's tile is split across BOTH rings
            # (halves the arrival latency), pair0 first, s before t
            fst, ftt = [], []
            for smp in range(SPC):
                fst.append(feats.tile([128, CSC, N], BF16, name=f"fs{smp}", tag="fs"))
                ftt.append(feats.tile([128, CTC, N], BF16, name=f"ft{smp}", tag="ft"))
            for smp in range(SPC):
                src_fs = fs_d.ap()[smp].rearrange("(c p) n -> p c n", p=128)
                nc.sync.dma_start(out=fst[smp][:, 0:3, :], in_=src_fs[:, 0:3, :])
                nc.scalar.dma_start(out=fst[smp][:, 3:CSC, :], in_=src_fs[:, 3:CSC, :])
                src_ft = ft_d.ap()[smp].rearrange("(c p) n -> p c n", p=128)
                nc.sync.dma_start(out=ftt[smp][:, 0:3, :], in_=src_ft[:, 0:3, :])
                nc.scalar.dma_start(out=ftt[smp][:, 3:CTC, :], in_=src_ft[:, 3:CTC, :])

            # per-partition exp bias: ln(N) on valid rows (folds the 1/N
            # marginals into K0'), -100 on pad rows so exp zeroes them
            pad_bias = {}
            for padp in sorted(set(PAD_P)):
                pb = singles.tile([128, 1], F32, name=f"padb{padp}")
                nc.vector.memset(pb, LN_N)
                nc.vector.memset(pb[96:128, :], -100.0)
                if padp > 96:
                    nc.vector.memset(pb[96:padp, :], LN_N)
                pad_bias[padp] = pb

            t3all = singles.tile([128, SPC], F32, name="t3all")
            ones = singles.tile([128, 1], F32, name="ones")
            nc.vector.memset(ones, 1.0)

            # ---- HAM warm-up: dummy matmuls keep the PE clock at 2.4GHz
            # through the DMA-bound head (zero real dependencies) ----
            zt = singles.tile([128, 512], BF16)
            nc.vector.memset(zt, 0.0)
            zp = psXP.tile([128, 512], F32, name="zp", tag="xp")
            for i in range(N_DUMMY):
                nc.tensor.matmul(zp, lhsT=zt[:, 0:128], rhs=zt,
                                 start=(i == 0), stop=(i == N_DUMMY - 1))

            S = [dict() for _ in range(SPC)]

            # ---- projection into a shared [128, N] psum tile: sample a in
            # col group [0:64), sample b in [64:128) (concurrent when both
            # emitted).  `halves` stages pair0 per-sample so the exp stream
            # starts as soon as sample 0's features land. ----
            def proj(pi, side, halves):
                st = S[2 * pi]
                key = f"xp{side}"
                if key not in st:
                    st[key] = psXP.tile([128, N], F32, name=f"xp{side}{pi}", tag="xp")
                xp = st[key]
                ftiles = fst if side == "s" else ftt
                wsb = wst_sb if side == "s" else wtt_sb
                nch = CSC if side == "s" else CTC
                for lo, hi in REG_N:
                    for c in range(nch):
                        for h in halves:
                            nc.tensor.matmul(
                                xp[64 * h:64 * h + 64, lo:hi], lhsT=wsb[:, c, :],
                                rhs=ftiles[2 * pi + h][:, c, lo:hi],
                                start=(c == 0), stop=(c == nch - 1))
                return xp

            # ---- per-half DVE/ACT chain: biases, squares/norms, scaled copies ----
            def dve_s(pi, halves):
                st = S[2 * pi]
                if "xsb" not in st:
                    st["xsb"] = pairs.tile([128, NP], F32, name=f"xsb{pi}", tag="xsb")
                    st["scr"] = pairs.tile([128, N], BF16, name=f"scr{pi}", tag="scr")
                    st["sss"] = small.tile([128, 1], F32, name=f"sss{pi}", tag="sm")
                xp_s = st["xps"]
                for h in halves:
                    sl = slice(64 * h, 64 * h + 64)
                    nc.vector.tensor_scalar_add(st["xsb"][sl, 0:N], in0=xp_s[sl, 0:N],
                                                scalar1=bs2[sl, :])
                    nc.vector.memset(st["xsb"][sl, N:NP], 0.0)
                    # ss_s = sum_n (xp+bs)^2 on the (idle-at-this-point) ACT engine
                    nc.scalar.activation(out=st["scr"][sl, :], in_=xp_s[sl, 0:N],
                                         func=AF.Square, bias=bs2[sl, :], scale=1.0,
                                         accum_out=st["sss"][sl, :])

            def dve_t(pi, halves):
                st = S[2 * pi]
                if "xts" not in st:
                    st["xts"] = pairs.tile([128, NP], BF16, name=f"xts{pi}", tag="xts")
                    st["sqt"] = pairs.tile([128, NP], BF16, name=f"sqt{pi}", tag="sqt")
                    st["sst"] = small.tile([128, 1], F32, name=f"sst{pi}", tag="sm")
                    st["rs2t"] = small.tile([128, 1], BF16, name=f"rs2t{pi}", tag="sm")
                    st["xss"] = pairs.tile([128, NP], BF16, name=f"xss{pi}", tag="xss")
                    for k in ("xts", "sqt", "rs2t", "xss"):
                        S[2 * pi + 1][k] = st[k]
                xp_t = st["xpt"]
                for h in halves:
                    sl = slice(64 * h, 64 * h + 64)
                    nc.vector.tensor_scalar_add(st["xts"][sl, 0:N], in0=xp_t[sl, 0:N],
                                                scalar1=bt2[sl, :])
                    nc.vector.memset(st["xts"][sl, N:NP], 0.0)
                    nc.scalar.activation(out=st["sqt"][sl, 0:N], in_=xp_t[sl, 0:N],
                                         func=AF.Square, bias=bt2[sl, :], scale=1.0,
                                         accum_out=st["sst"][sl, :])
                    nc.vector.memset(st["sqt"][sl, N:NP], 0.0)
                    # rst = 1/sqrt(ss_s*ss_t) folds both L2 norms into xss
                    m64 = small.tile([64, 1], F32, name=f"m64{pi}_{h}", tag="sm")
                    nc.vector.tensor_mul(m64, st["sss"][sl, :], st["sst"][sl, :])
                    lnm = small.tile([64, 1], F32, name=f"lnm{pi}_{h}", tag="sm")
                    nc.scalar.activation(out=lnm, in_=m64, func=AF.Ln)
                    rst = small.tile([64, 1], F32, name=f"rst{pi}_{h}", tag="sm")
                    nc.scalar.activation(out=rst, in_=lnm, func=AF.Exp, scale=-0.5)
                    with nc.allow_low_precision(reason="bf16 stationaries validated"):
                        nc.vector.reciprocal(out=st["rs2t"][sl, :], in_=st["sst"][sl, :])
                    nc.vector.tensor_scalar_mul(st["xss"][sl, :], in0=st["xsb"][sl, :],
                                                scalar1=rst)

            # ---- K-gen: dps (dots) -> exp -> k tile; row-packed when both
            # halves emitted together; the k0 side fuses g2 = k0 o dots ----
            def kgen(pi, kind, halves):
                st = S[2 * pi]
                lh = st["xts"] if kind == "k0t" else st["xss"]
                rh = st["xss"] if kind == "k0t" else st["xts"]
                width = NP if kind == "k0t" else N
                regs = REG_FULL if kind == "k0t" else REG_N
                for h in halves:
                    smp = 2 * pi + h
                    S[smp][kind] = ktp.tile([128, NB, width], BF16,
                                            name=f"{kind}{smp}", tag=kind)
                    if kind == "k0":
                        S[smp]["g2"] = ktp.tile([128, NB, N], BF16,
                                                name=f"g2{smp}", tag="g2")
                for blk in range(NB):
                    pb = pad_bias[PAD_P[blk]]
                    dps = {}
                    for h in halves:
                        dps[h] = psD.tile([128, width], F32,
                                          name=f"d{kind}{2 * pi + h}_{blk}", tag="d")
                    for lo, hi in regs:
                        for h in halves:
                            sl = slice(64 * h, 64 * h + 64)
                            nc.tensor.matmul(dps[h][:, lo:hi],
                                             lhsT=lh[sl, blk:NP:NB],
                                             rhs=rh[sl, lo:hi])
                    for h in halves:
                        smp = 2 * pi + h
                        nc.scalar.activation(out=S[smp][kind][:, blk, :], in_=dps[h],
                                             func=AF.Exp, scale=-2.0 / REG, bias=pb)
                        if kind == "k0":
                            nc.vector.tensor_mul(S[smp]["g2"][:, blk, :],
                                                 S[smp][kind][:, blk, :], dps[h])

            # ---- q0 = exp(sq_t/reg) as [128, NB] columns; both samples of a
            # pair run concurrently (col groups -> rows 0 / 32 of one tile) ----
            def q0_prep(pi):
                a, b = 2 * pi, 2 * pi + 1
                st = S[a]
                row = psB.tile([33, NP], F32, name=f"q0r{pi}", tag="row")
                for lo, hi in REG_FULL:
                    nc.tensor.matmul(row[0:1, lo:hi], lhsT=st["rs2t"][0:HID, 0:1],
                                     rhs=st["sqt"][0:HID, lo:hi])
                    nc.tensor.matmul(row[32:33, lo:hi], lhsT=st["rs2t"][HID:128, 0:1],
                                     rhs=st["sqt"][HID:128, lo:hi])
                rsb = rows.tile([33, NP], F32, name=f"q0sb{pi}", tag="qrow")
                nc.vector.tensor_copy(out=rsb, in_=row[0:33, :])
                for smp, r in ((a, 0), (b, 32)):
                    qf = cols.tile([128, NB], F32, name=f"q0f{smp}", tag="colF")
                    dmaq(smp).dma_start(
                        out=qf, in_=rsb[r:r + 1, :].rearrange("o (p b) -> o p b", b=NB))
                    qc = cols.tile([128, NB], BF16, name=f"q0c{smp}", tag="colB")
                    nc.scalar.activation(out=qc, in_=qf, func=AF.Exp, scale=1.0 / REG)
                    S[smp]["q0"] = qc

            # ---- p-half: p = 1/(K0' q0) via the k0t moving stream; the two
            # samples of a pair run concurrently in col groups ----
            def p_half(pi):
                a, b = 2 * pi, 2 * pi + 1
                row = psB.tile([33, NP], F32, name=f"pr{pi}", tag="row")
                for lo, hi in REG_FULL:
                    for blk in range(NB):
                        nc.tensor.matmul(row[0:1, lo:hi],
                                         lhsT=S[a]["q0"][:, blk:blk + 1],
                                         rhs=S[a]["k0t"][:, blk, lo:hi],
                                         start=(blk == 0), stop=(blk == NB - 1))
                        nc.tensor.matmul(row[32:33, lo:hi],
                                         lhsT=S[b]["q0"][:, blk:blk + 1],
                                         rhs=S[b]["k0t"][:, blk, lo:hi],
                                         start=(blk == 0), stop=(blk == NB - 1))
                rsb = rows.tile([33, NP], F32, name=f"prsb{pi}", tag="prow")
                nc.vector.tensor_copy(out=rsb, in_=row[0:33, :])
                for smp, r in ((a, 0), (b, 32)):
                    pf = cols.tile([128, NB], F32, name=f"pf{smp}", tag="colF")
                    dmaq(smp).dma_start(
                        out=pf, in_=rsb[r:r + 1, :].rearrange("o (p b) -> o p b", b=NB))
                    pc = cols.tile([128, NB], BF16, name=f"pc{smp}", tag="colB")
                    with nc.allow_low_precision(reason="bf16 stationaries validated"):
                        nc.vector.reciprocal(out=pc, in_=pf)
                    S[smp]["p"] = pc

            # ---- u = K0'^T p and w = (K0' o dots)^T p, col-packed into one
            # psum tile (rows at partitions 0 and 32); term3 = -2/N sum w/u ----
            def uw_final(smp):
                st = S[smp]
                uw = psB.tile([33, NP], F32, name=f"uw{smp}", tag="row")
                for lo, hi in REG_N:
                    for blk in range(NB):
                        nc.tensor.matmul(uw[0:1, lo:hi],
                                         lhsT=st["p"][:, blk:blk + 1],
                                         rhs=st["k0"][:, blk, lo:hi],
                                         start=(blk == 0), stop=(blk == NB - 1))
                        nc.tensor.matmul(uw[32:33, lo:hi],
                                         lhsT=st["p"][:, blk:blk + 1],
                                         rhs=st["g2"][:, blk, lo:hi],
                                         start=(blk == 0), stop=(blk == NB - 1))
                # rows -> SBUF (33-lane copy), pads: u=1 / w=0 so pad cols
                # contribute exactly 0 after the division
                uwsb = rows.tile([33, NP], F32, name=f"uwsb{smp}", tag="uwrow")
                nc.vector.tensor_copy(out=uwsb[0:33, 0:N], in_=uw[0:33, 0:N])
                nc.vector.memset(uwsb[0:1, N:NP], 1.0)
                nc.vector.memset(uwsb[32:33, N:NP], 0.0)
                ucol = cols.tile([128, NB], F32, name=f"uc{smp}", tag="colF")
                dmaq(smp).dma_start(
                    out=ucol, in_=uwsb[0:1, :].rearrange("o (p b) -> o p b", b=NB))
                wcol = cols.tile([128, NB], F32, name=f"wc{smp}", tag="colW")
                dmaq(smp + 1).dma_start(
                    out=wcol, in_=uwsb[32:33, :].rearrange("o (p b) -> o p b", b=NB))
                qcol = cols.tile([128, NB], F32, name=f"qc{smp}", tag="colQ")
                nc.vector.reciprocal(out=qcol, in_=ucol)
                t3c = cols.tile([128, NB], F32, name=f"t3c{smp}", tag="colT")
                nc.vector.tensor_mul(t3c, wcol, qcol)
                nc.vector.tensor_reduce(t3all[:, smp:smp + 1], t3c, axis=AX, op=OP.add)

            # ---- emission order (engine queues are in-order; this order
            # keeps the ScalarE exp stream as gap-free as possible).
            # pair0 is staged per-sample so the first exps start as soon as
            # sample 0's features land. ----
            proj(0, "s", [0]); dve_s(0, [0])
            proj(0, "t", [0]); dve_t(0, [0])
            kgen(0, "k0t", [0])
            proj(0, "s", [1]); dve_s(0, [1])
            proj(0, "t", [1]); dve_t(0, [1])
            kgen(0, "k0t", [1])
            q0_prep(0)
            kgen(0, "k0", [0, 1])
            p_half(0)
            proj(1, "s", [0, 1]); dve_s(1, [0, 1])
            proj(1, "t", [0, 1]); dve_t(1, [0, 1])
            kgen(1, "k0t", [0, 1])
            q0_prep(1)
            uw_final(0)
            uw_final(1)
            p_half(1)
            kgen(1, "k0", [0, 1])
            uw_final(2)
            uw_final(3)

            # partition-sum the 4 per-sample columns with one ones-matmul
            fin = psXP.tile([1, SPC], F32, name="fin", tag="xp")
            nc.tensor.matmul(fin, lhsT=ones, rhs=t3all)
            t3s = singles.tile([1, 1], F32, name="t3s")
            nc.vector.tensor_reduce(t3s, fin, axis=AX, op=OP.add)
            loss_sb = singles.tile([1, 1], F32, name="loss_sb")
            nc.vector.tensor_scalar_mul(loss_sb, in0=t3s,
                                        scalar1=-2.0 / (N * BS))
            nc.sync.dma_start(out=loss_d.ap().rearrange("(p o) -> p o", o=1),
                              in_=loss_sb)

    return nc


_CACHED_NC = None


def _get_nc():
    global _CACHED_NC
    if _CACHED_NC is None:
        _CACHED_NC = build_program()
    return _CACHED_NC


TERM12 = 2.0 * HID / N     # term1 + term2 are analytic (L2norm over tokens)


def run(inputs, trace=False, **trace_kwargs):
    import ml_dtypes
    bf = ml_dtypes.bfloat16
    feat_s = np.ascontiguousarray(
        np.asarray(inputs["feat_s"], dtype=np.float32).reshape(BS, CS, N).astype(bf))
    feat_t = np.ascontiguousarray(
        np.asarray(inputs["feat_t"], dtype=np.float32).reshape(BS, CT, N).astype(bf))
    wst = np.ascontiguousarray(np.asarray(inputs["Ws"], dtype=np.float32).T.astype(bf))
    wtt = np.ascontiguousarray(np.asarray(inputs["Wt"], dtype=np.float32).T.astype(bf))
    bs_ = np.ascontiguousarray(np.asarray(inputs["bs"], dtype=np.float32))
    bt_ = np.ascontiguousarray(np.asarray(inputs["bt"], dtype=np.float32))

    in_maps = []
    for i in range(N_CORES):
        in_maps.append({
            "feat_s": np.ascontiguousarray(feat_s[i * SPC:(i + 1) * SPC]),
            "feat_t": np.ascontiguousarray(feat_t[i * SPC:(i + 1) * SPC]),
            "WsT": wst, "WtT": wtt, "bs": bs_, "bt": bt_,
        })

    nc = _get_nc()
    res = run_bass_kernel_spmd(nc, in_maps, list(range(N_CORES)),
                               trace=trace, **trace_kwargs)
    total = sum(float(res.results[i]["loss"][0]) for i in range(N_CORES))
    return np.float32(TERM12 + total), res


def kernel(**inputs) -> np.ndarray:
    out, _ = run(inputs)
    return np.asarray(out, dtype=np.float32)
